# revision 29
# baseline (speedup 1.0000x reference)
"""Trainium2 Bass kernel for nn_BoundingBoxExtractor.

Math (from the reference): per (batch, channel) image, with
binary = (mask >= 0.5):
  x_min = y_min = 1 if ALL pixels on else 0
  x_max = (last column with any on pixel) + 1, or 0 if none
  y_max = (last row with any on pixel) + 1, or 0 if none

Device mapping (per core, 4 batches x 8 channels = 32 images of 512x512):
  - DMA each image as a [128, 4, 512] f32 tile (partition = row % 128).
  - DVE tensor_scalar(is_ge 0.5) per 128-row chunk -> binary bf16 tile AND,
    via accum_out(add), the per-row on-count rc. rc>=0.5 gives row any-on
    (y side), rc>=511.5 gives row all-on (the x_min/y_min test).
  - PE matmul ones[128,1]^T @ binary[128,512], accumulating the 4 chunks into
    a PSUM [1, 512] tile = per-column on-counts.
  - ScalarE copies each image's column counts to partition 0 of a wide
    staging row (engines are lane-locked); one SBUF->SBUF DMA redistributes
    to [32, 512] for batched post-processing.
  - Tail: batched DVE ops build a [128, 96] stats tile (notall / xmax-part /
    ymax-part per image); PE transpose + DVE max-reduce does the
    cross-partition max; a [96, 1] result goes out.

Scheduling notes (hardware sync-wait limits): DVE instructions can carry only
ONE sync wait, DMA ~2. 1-element DVE "probes" absorb the DMA-completion and
PE ticks before the real ops run, and mask_pool bufs=8 keeps each load-DMA's
WAW predecessor on the SAME DMA lane (8 lanes, round-robin) so no cross-lane
wait is needed.
"""

import sys

import numpy as np

try:
    import concourse.bass as bass  # noqa: F401
except ImportError:  # container default: repo lives in /opt/trn_rl_repo
    sys.path.insert(0, "/opt/trn_rl_repo")
    import concourse.bass as bass

import concourse.bacc as bacc
import concourse.tile as tile
from concourse import mybir
from concourse.bass_utils import run_bass_kernel_spmd

N_CORES = 8
B, C, H, W = 32, 8, 512, 512
BPC = B // N_CORES  # batches per core
IMGS = BPC * C  # images per core
NCHUNK = H // 128  # 128-row chunks per image
SCALE = 2.0

F32 = mybir.dt.float32
BF16 = mybir.dt.bfloat16


def build_bass():
    # Bacc (not raw Bass): its generate_event_semaphores pass splits
    # multi-semaphore waits to satisfy the 1-wait-per-instruction HW limit.
    nc = bacc.Bacc("TRN2", target_bir_lowering=False, debug=False)

    mask = nc.dram_tensor("mask", [BPC, C, H, W], F32, kind="ExternalInput").ap()
    # x_iota[i, w] = w + 1
    xio = nc.dram_tensor("xio", [IMGS, W], F32, kind="ExternalInput").ap()
    # y_iota[p, j] = (j % NCHUNK) * 128 + p + 1  (global row index + 1)
    yio = nc.dram_tensor("yio", [128, IMGS * NCHUNK], F32, kind="ExternalInput").ap()
    ident = nc.dram_tensor("ident", [128, 128], F32, kind="ExternalInput").ap()
    # stats col: [0:IMGS) = not_all_on, [IMGS:2*IMGS) = y_max
    stats_out = nc.dram_tensor("stats", [2 * IMGS, 1], F32, kind="ExternalOutput").ap()
    xmax_out = nc.dram_tensor("xmax", [IMGS, 1], F32, kind="ExternalOutput").ap()

    imgs = mask.rearrange("b c h w -> (b c) h w")

    with tile.TileContext(nc) as tc:
        with (
            tc.tile_pool(name="consts", bufs=1) as consts,
            tc.tile_pool(name="mask_pool", bufs=8) as mask_pool,
            tc.tile_pool(name="bin_pool", bufs=4) as bin_pool,
            tc.tile_pool(name="acc", bufs=1) as acc,
            tc.tile_pool(name="psum_mm", bufs=7, space="PSUM") as psum_mm,
            tc.tile_pool(name="psum_t", bufs=1, space="PSUM") as psum_t,
        ):
            ones = consts.tile([128, 1], BF16)
            nc.vector.memset(ones, 1.0)
            xio_t = consts.tile([IMGS, W], F32)
            nc.sync.dma_start(out=xio_t, in_=xio)
            yio_t = consts.tile([128, IMGS * NCHUNK], F32)
            nc.sync.dma_start(out=yio_t, in_=yio)
            ident_t = consts.tile([128, 128], F32)
            nc.sync.dma_start(out=ident_t, in_=ident)
            rc = acc.tile([128, IMGS * NCHUNK], F32)  # per-row on-counts
            xwide = acc.tile([1, IMGS * W], F32)  # col counts staging (part 0)

            ps_hist = []
            for i in range(IMGS):
                img = imgs[i].rearrange("(c p) w -> p c w", c=NCHUNK)
                xt = mask_pool.tile([128, NCHUNK, W], F32)
                nc.sync.dma_start(out=xt, in_=img)
                bt = bin_pool.tile([128, NCHUNK, W], BF16)
                for c in range(NCHUNK):
                    j = i * NCHUNK + c
                    nc.vector.tensor_scalar(
                        out=bt[:, c, :],
                        in0=xt[:, c, :],
                        scalar1=0.5,
                        scalar2=None,
                        op0=mybir.AluOpType.is_ge,
                        op1=mybir.AluOpType.add,
                        accum_out=rc[:, j : j + 1],
                    )
                ps = psum_mm.tile([1, W], F32)
                ps_hist.append(ps)
                for c in range(NCHUNK):
                    nc.tensor.matmul(
                        ps,
                        ones,
                        bt[:, c, :],
                        start=(c == 0),
                        stop=(c == NCHUNK - 1),
                    )
                nc.scalar.activation(
                    out=xwide[0:1, i * W : (i + 1) * W],
                    in_=ps,
                    func=mybir.ActivationFunctionType.Copy,
                )

            # ---- batched tail ----
            xstack = acc.tile([IMGS, W], F32)
            nc.sync.dma_start(out=xstack, in_=xwide)
            stat = acc.tile([128, 2 * IMGS], F32)

            # not-all-on per (partition, image): max over chunks of (rc <= 511.5)
            nr = acc.tile([128, IMGS * NCHUNK], F32)
            nc.vector.tensor_scalar(
                out=nr,
                in0=rc,
                scalar1=float(W) - 0.5,
                scalar2=None,
                op0=mybir.AluOpType.is_le,
            )
            nc.vector.tensor_reduce(
                out=stat[:, 0:IMGS],
                in_=nr.rearrange("p (i c) -> p i c", c=NCHUNK),
                axis=mybir.AxisListType.X,
                op=mybir.AluOpType.max,
            )
            # x_max per image from column counts (partitions 0..31):
            # (count >= 0.5) * (w + 1), then free-dim max
            xon = acc.tile([IMGS, W], F32)
            nc.vector.tensor_scalar(
                out=xon,
                in0=xstack,
                scalar1=0.5,
                scalar2=None,
                op0=mybir.AluOpType.is_ge,
            )
            xcon = acc.tile([IMGS, W], F32)
            nc.vector.tensor_mul(xcon, xon, xio_t)
            xmax_t = acc.tile([IMGS, 1], F32)
            nc.vector.tensor_reduce(
                out=xmax_t,
                in_=xcon,
                axis=mybir.AxisListType.X,
                op=mybir.AluOpType.max,
            )
            # y contribution per (partition, image): (row any on) * (row + 1)
            yon = acc.tile([128, IMGS * NCHUNK], F32)
            nc.vector.tensor_scalar(
                out=yon,
                in0=rc,
                scalar1=0.5,
                scalar2=None,
                op0=mybir.AluOpType.is_ge,
            )
            ymul = acc.tile([128, IMGS * NCHUNK], F32)
            nc.vector.tensor_mul(ymul, yon, yio_t)
            nc.vector.tensor_reduce(
                out=stat[:, IMGS : 2 * IMGS],
                in_=ymul.rearrange("p (i c) -> p i c", c=NCHUNK),
                axis=mybir.AxisListType.X,
                op=mybir.AluOpType.max,
            )
            # x_max is already per-image ([IMGS, 1] on partitions 0..31) - it
            # goes straight out. notall/y_max need the cross-partition max:
            # transpose [128, 64] -> PSUM [64, 128], then free-dim max-reduce.
            nc.sync.dma_start(out=xmax_out, in_=xmax_t)
            pt = psum_t.tile([2 * IMGS, 128], F32)
            nc.tensor.transpose(pt, stat, ident_t)
            statr = acc.tile([2 * IMGS, 1], F32)
            nc.vector.tensor_reduce(
                out=statr,
                in_=pt,
                axis=mybir.AxisListType.X,
                op=mybir.AluOpType.max,
            )
            nc.sync.dma_start(out=stats_out, in_=statr)

    nc.finalize()
    return nc


_NC = None


def _get_nc():
    global _NC
    if _NC is None:
        _NC = build_bass()
    return _NC


_RUNNER = None


def _make_runner():
    """Builds a reusable sharded-jit runner for the SPMD kernel (the same
    lowering run_bass_via_pjrt does, but cached so repeat calls reuse the
    compiled executable and device-resident inputs are possible)."""
    global _RUNNER
    if _RUNNER is not None:
        return _RUNNER
    import jax
    from concourse import bass2jax, mybir as mb

    nc = _get_nc()
    bass2jax.install_neuronx_cc_hook()
    partition_name = nc.partition_id_tensor.name if nc.partition_id_tensor else None
    in_names, out_names, out_avals, zero_outs = [], [], [], []
    for alloc in nc.m.functions[0].allocations:
        if not isinstance(alloc, mb.MemoryLocationSet):
            continue
        name = alloc.memorylocations[0].name
        if alloc.kind == "ExternalInput":
            if name != partition_name:
                in_names.append(name)
        elif alloc.kind == "ExternalOutput":
            out_names.append(name)
            shape = tuple(alloc.tensor_shape)
            dtype = mb.dt.np(alloc.dtype)
            out_avals.append(jax.core.ShapedArray(shape, dtype))
            zero_outs.append(np.zeros(shape, dtype))
    n_params = len(in_names)
    all_in_names = list(in_names) + list(out_names)
    if partition_name is not None:
        all_in_names.append(partition_name)
    donate = tuple(range(n_params, n_params + len(out_names)))

    def _body(*args):
        operands = list(args)
        if partition_name is not None:
            operands.append(bass2jax.partition_id_tensor())
        outs = bass2jax._bass_exec_p.bind(
            *operands,
            out_avals=tuple(out_avals),
            in_names=tuple(all_in_names),
            out_names=tuple(out_names),
            lowering_input_output_aliases=(),
            sim_require_finite=True,
            sim_require_nnan=True,
            nc=nc,
        )
        return tuple(outs)

    devices = jax.devices()[:N_CORES]
    mesh = bass2jax.Mesh(np.asarray(devices), ("core",))
    P = bass2jax.PartitionSpec
    sharded = jax.jit(
        bass2jax.shard_map(
            _body,
            mesh=mesh,
            in_specs=(P("core"),) * (n_params + len(out_names)),
            out_specs=(P("core"),) * len(out_names),
            check_rep=False,
        ),
        donate_argnums=donate,
        keep_unused=True,
    )
    _RUNNER = (sharded, mesh, in_names, out_names, out_avals, zero_outs)
    return _RUNNER


def _concat_inputs(in_maps, in_names):
    return [
        np.concatenate([np.asarray(m[name]) for m in in_maps], axis=0)
        for name in in_names
    ]


def _split_outputs(out_arrs, out_names, out_avals):
    results = []
    for c in range(N_CORES):
        results.append(
            {
                name: np.asarray(out_arrs[i]).reshape(
                    N_CORES, *out_avals[i].shape
                )[c]
                for i, name in enumerate(out_names)
            }
        )
    return results


def _const_inputs():
    xio = np.broadcast_to(
        np.arange(1, W + 1, dtype=np.float32)[None, :], (IMGS, W)
    ).copy()
    p = np.arange(128, dtype=np.float32)[:, None]
    c = np.arange(IMGS * NCHUNK, dtype=np.float32)[None, :] % NCHUNK
    yio = (c * 128.0 + p + 1.0).astype(np.float32)
    ident = np.eye(128, dtype=np.float32)
    return xio, yio, ident


def _in_maps_for(mask):
    xio, yio, ident = _const_inputs()
    in_maps = []
    for k in range(N_CORES):
        shard = np.ascontiguousarray(mask[k * BPC : (k + 1) * BPC])
        in_maps.append({"mask": shard, "xio": xio, "yio": yio, "ident": ident})
    return in_maps


def _extract(results):
    stats = np.stack(
        [results[k]["stats"].reshape(2, IMGS) for k in range(N_CORES)]
    )  # [cores, 2, IMGS]
    notall = stats[:, 0].reshape(B, C)
    ymax = stats[:, 1].reshape(B, C)
    xmax = np.stack(
        [results[k]["xmax"].reshape(IMGS) for k in range(N_CORES)]
    ).reshape(B, C)
    return notall, xmax, ymax


def run_device(mask, trace=False, **kw):
    """Runs the SPMD kernel; returns (not_all_on, x_max, y_max) as [B, C] f32
    arrays plus the per-core results."""
    if trace:
        nc = _get_nc()
        res = run_bass_kernel_spmd(
            nc, _in_maps_for(mask), core_ids=list(range(N_CORES)), trace=True, **kw
        )
        return (*_extract(res.results), res)
    sharded, mesh, in_names, out_names, out_avals, zero_outs = _make_runner()
    concat_in = _concat_inputs(_in_maps_for(mask), in_names)
    concat_zeros = [
        np.zeros((N_CORES * z.shape[0], *z.shape[1:]), z.dtype) for z in zero_outs
    ]
    out_arrs = sharded(*concat_in, *concat_zeros)
    results = _split_outputs(out_arrs, out_names, out_avals)
    return (*_extract(results), None)


def kernel(mask):
    mask = np.ascontiguousarray(np.asarray(mask, dtype=np.float32))
    assert mask.shape == (B, C, H, W), mask.shape
    notall, x_max, y_max, _ = run_device(mask)

    all_on = (notall == 0.0).astype(np.float32)
    x_min = all_on
    y_min = all_on
    object_found = ((y_max > y_min) & (x_max > x_min)).astype(np.int32)[..., None]
    boxes_scaled = np.stack([x_min, y_min, x_max, y_max], axis=-1).astype(np.float32)
    boxes = (boxes_scaled * SCALE).astype(np.float32)
    return object_found, boxes_scaled, boxes


# revision 34
# speedup vs baseline: 1330.3910x; 1330.3910x over previous
"""Trainium2 Bass kernel for nn_BoundingBoxExtractor.

Math (from the reference): per (batch, channel) image, with
binary = (mask >= 0.5):
  x_min = y_min = 1 if ALL pixels on else 0
  x_max = (last column with any on pixel) + 1, or 0 if none
  y_max = (last row with any on pixel) + 1, or 0 if none

Device mapping (per core, 4 batches x 8 channels = 32 images of 512x512):
  - DMA each image as a [128, 4, 512] f32 tile (partition = row % 128).
  - DVE tensor_scalar(is_ge 0.5) per 128-row chunk -> binary bf16 tile AND,
    via accum_out(add), the per-row on-count rc. rc>=0.5 gives row any-on
    (y side), rc>=511.5 gives row all-on (the x_min/y_min test).
  - PE matmul ones[128,1]^T @ binary[128,512], accumulating the 4 chunks into
    a PSUM [1, 512] tile = per-column on-counts.
  - ScalarE copies each image's column counts to partition 0 of a wide
    staging row (engines are lane-locked); one SBUF->SBUF DMA redistributes
    to [32, 512] for batched post-processing.
  - Tail: batched DVE ops build a [128, 96] stats tile (notall / xmax-part /
    ymax-part per image); PE transpose + DVE max-reduce does the
    cross-partition max; a [96, 1] result goes out.

Scheduling notes (hardware sync-wait limits): DVE instructions can carry only
ONE sync wait, DMA ~2. 1-element DVE "probes" absorb the DMA-completion and
PE ticks before the real ops run, and mask_pool bufs=8 keeps each load-DMA's
WAW predecessor on the SAME DMA lane (8 lanes, round-robin) so no cross-lane
wait is needed.
"""

import sys

import numpy as np

try:
    import concourse.bass as bass  # noqa: F401
except ImportError:  # container default: repo lives in /opt/trn_rl_repo
    sys.path.insert(0, "/opt/trn_rl_repo")
    import concourse.bass as bass

import concourse.bacc as bacc
import concourse.tile as tile
from concourse import mybir
from concourse.bass_utils import run_bass_kernel_spmd

N_CORES = 8
B, C, H, W = 32, 8, 512, 512
BPC = B // N_CORES  # batches per core
IMGS = BPC * C  # images per core
NCHUNK = H // 128  # 128-row chunks per image
SCALE = 2.0

F32 = mybir.dt.float32
BF16 = mybir.dt.bfloat16


def build_bass(repeat=1):
    # Bacc (not raw Bass): its generate_event_semaphores pass splits
    # multi-semaphore waits to satisfy the 1-wait-per-instruction HW limit.
    # repeat>1 re-runs the whole main loop (identical results) — used only to
    # measure per-iteration device time by slope, since the axon tunnel hides
    # single-kernel wall time behind transfer costs.
    nc = bacc.Bacc("TRN2", target_bir_lowering=False, debug=False)

    mask = nc.dram_tensor("mask", [BPC, C, H, W], F32, kind="ExternalInput").ap()
    # x_iota[i, w] = w + 1
    xio = nc.dram_tensor("xio", [IMGS, W], F32, kind="ExternalInput").ap()
    # y_iota[p, j] = (j % NCHUNK) * 128 + p + 1  (global row index + 1)
    yio = nc.dram_tensor("yio", [128, IMGS * NCHUNK], F32, kind="ExternalInput").ap()
    ident = nc.dram_tensor("ident", [128, 128], F32, kind="ExternalInput").ap()
    # stats col: [0:IMGS) = not_all_on, [IMGS:2*IMGS) = y_max
    stats_out = nc.dram_tensor("stats", [2 * IMGS, 1], F32, kind="ExternalOutput").ap()
    xmax_out = nc.dram_tensor("xmax", [IMGS, 1], F32, kind="ExternalOutput").ap()

    imgs = mask.rearrange("b c h w -> (b c) h w")

    with tile.TileContext(nc) as tc:
        with (
            tc.tile_pool(name="consts", bufs=1) as consts,
            tc.tile_pool(name="mask_pool", bufs=8) as mask_pool,
            tc.tile_pool(name="bin_pool", bufs=4) as bin_pool,
            tc.tile_pool(name="acc", bufs=1) as acc,
            tc.tile_pool(name="psum_mm", bufs=7, space="PSUM") as psum_mm,
            tc.tile_pool(name="psum_t", bufs=1, space="PSUM") as psum_t,
        ):
            ones = consts.tile([128, 1], BF16)
            nc.vector.memset(ones, 1.0)
            xio_t = consts.tile([IMGS, W], F32)
            nc.sync.dma_start(out=xio_t, in_=xio)
            yio_t = consts.tile([128, IMGS * NCHUNK], F32)
            nc.sync.dma_start(out=yio_t, in_=yio)
            ident_t = consts.tile([128, 128], F32)
            nc.sync.dma_start(out=ident_t, in_=ident)
            rc = acc.tile([128, IMGS * NCHUNK], F32)  # per-row on-counts
            xwide = acc.tile([1, IMGS * W], F32)  # col counts staging (part 0)

            ps_hist = []
            for i_rep in range(repeat * IMGS):
                i = i_rep % IMGS
                img = imgs[i].rearrange("(c p) w -> p c w", c=NCHUNK)
                xt = mask_pool.tile([128, NCHUNK, W], F32)
                nc.sync.dma_start(out=xt, in_=img)
                bt = bin_pool.tile([128, NCHUNK, W], BF16)
                for c in range(NCHUNK):
                    j = i * NCHUNK + c
                    nc.vector.tensor_scalar(
                        out=bt[:, c, :],
                        in0=xt[:, c, :],
                        scalar1=0.5,
                        scalar2=None,
                        op0=mybir.AluOpType.is_ge,
                        op1=mybir.AluOpType.add,
                        accum_out=rc[:, j : j + 1],
                    )
                ps = psum_mm.tile([1, W], F32)
                ps_hist.append(ps)  # noqa: list grows with repeats; unused
                for c in range(NCHUNK):
                    nc.tensor.matmul(
                        ps,
                        ones,
                        bt[:, c, :],
                        start=(c == 0),
                        stop=(c == NCHUNK - 1),
                    )
                nc.scalar.activation(
                    out=xwide[0:1, i * W : (i + 1) * W],
                    in_=ps,
                    func=mybir.ActivationFunctionType.Copy,
                )

            # ---- batched tail ----
            xstack = acc.tile([IMGS, W], F32)
            nc.sync.dma_start(out=xstack, in_=xwide)
            stat = acc.tile([128, 2 * IMGS], F32)

            # not-all-on per (partition, image): max over chunks of (rc <= 511.5)
            nr = acc.tile([128, IMGS * NCHUNK], F32)
            nc.vector.tensor_scalar(
                out=nr,
                in0=rc,
                scalar1=float(W) - 0.5,
                scalar2=None,
                op0=mybir.AluOpType.is_le,
            )
            nc.vector.tensor_reduce(
                out=stat[:, 0:IMGS],
                in_=nr.rearrange("p (i c) -> p i c", c=NCHUNK),
                axis=mybir.AxisListType.X,
                op=mybir.AluOpType.max,
            )
            # x_max per image from column counts (partitions 0..31):
            # (count >= 0.5) * (w + 1), then free-dim max
            xon = acc.tile([IMGS, W], F32)
            nc.vector.tensor_scalar(
                out=xon,
                in0=xstack,
                scalar1=0.5,
                scalar2=None,
                op0=mybir.AluOpType.is_ge,
            )
            xcon = acc.tile([IMGS, W], F32)
            nc.vector.tensor_mul(xcon, xon, xio_t)
            xmax_t = acc.tile([IMGS, 1], F32)
            nc.vector.tensor_reduce(
                out=xmax_t,
                in_=xcon,
                axis=mybir.AxisListType.X,
                op=mybir.AluOpType.max,
            )
            # y contribution per (partition, image): (row any on) * (row + 1)
            yon = acc.tile([128, IMGS * NCHUNK], F32)
            nc.vector.tensor_scalar(
                out=yon,
                in0=rc,
                scalar1=0.5,
                scalar2=None,
                op0=mybir.AluOpType.is_ge,
            )
            ymul = acc.tile([128, IMGS * NCHUNK], F32)
            nc.vector.tensor_mul(ymul, yon, yio_t)
            nc.vector.tensor_reduce(
                out=stat[:, IMGS : 2 * IMGS],
                in_=ymul.rearrange("p (i c) -> p i c", c=NCHUNK),
                axis=mybir.AxisListType.X,
                op=mybir.AluOpType.max,
            )
            # x_max is already per-image ([IMGS, 1] on partitions 0..31) - it
            # goes straight out. notall/y_max need the cross-partition max:
            # transpose [128, 64] -> PSUM [64, 128], then free-dim max-reduce.
            nc.sync.dma_start(out=xmax_out, in_=xmax_t)
            pt = psum_t.tile([2 * IMGS, 128], F32)
            nc.tensor.transpose(pt, stat, ident_t)
            statr = acc.tile([2 * IMGS, 1], F32)
            nc.vector.tensor_reduce(
                out=statr,
                in_=pt,
                axis=mybir.AxisListType.X,
                op=mybir.AluOpType.max,
            )
            nc.sync.dma_start(out=stats_out, in_=statr)

    nc.finalize()
    return nc


_NC = {}


def _get_nc(repeat=1):
    if repeat not in _NC:
        _NC[repeat] = build_bass(repeat)
    return _NC[repeat]


_RUNNER = {}


def _make_runner(repeat=1):
    """Builds a reusable sharded-jit runner for the SPMD kernel (the same
    lowering run_bass_via_pjrt does, but cached so repeat calls reuse the
    compiled executable and device-resident inputs are possible)."""
    if repeat in _RUNNER:
        return _RUNNER[repeat]
    import jax
    from concourse import bass2jax, mybir as mb

    nc = _get_nc(repeat)
    bass2jax.install_neuronx_cc_hook()
    partition_name = nc.partition_id_tensor.name if nc.partition_id_tensor else None
    in_names, out_names, out_avals, zero_outs = [], [], [], []
    for alloc in nc.m.functions[0].allocations:
        if not isinstance(alloc, mb.MemoryLocationSet):
            continue
        name = alloc.memorylocations[0].name
        if alloc.kind == "ExternalInput":
            if name != partition_name:
                in_names.append(name)
        elif alloc.kind == "ExternalOutput":
            out_names.append(name)
            shape = tuple(alloc.tensor_shape)
            dtype = mb.dt.np(alloc.dtype)
            out_avals.append(jax.core.ShapedArray(shape, dtype))
            zero_outs.append(np.zeros(shape, dtype))
    n_params = len(in_names)
    all_in_names = list(in_names) + list(out_names)
    if partition_name is not None:
        all_in_names.append(partition_name)
    donate = tuple(range(n_params, n_params + len(out_names)))

    def _body(*args):
        operands = list(args)
        if partition_name is not None:
            operands.append(bass2jax.partition_id_tensor())
        outs = bass2jax._bass_exec_p.bind(
            *operands,
            out_avals=tuple(out_avals),
            in_names=tuple(all_in_names),
            out_names=tuple(out_names),
            lowering_input_output_aliases=(),
            sim_require_finite=True,
            sim_require_nnan=True,
            nc=nc,
        )
        return tuple(outs)

    devices = jax.devices()[:N_CORES]
    mesh = bass2jax.Mesh(np.asarray(devices), ("core",))
    P = bass2jax.PartitionSpec
    sharded = jax.jit(
        bass2jax.shard_map(
            _body,
            mesh=mesh,
            in_specs=(P("core"),) * (n_params + len(out_names)),
            out_specs=(P("core"),) * len(out_names),
            check_rep=False,
        ),
        donate_argnums=donate,
        keep_unused=True,
    )
    _RUNNER[repeat] = (sharded, mesh, in_names, out_names, out_avals, zero_outs)
    return _RUNNER[repeat]


def _concat_inputs(in_maps, in_names):
    return [
        np.concatenate([np.asarray(m[name]) for m in in_maps], axis=0)
        for name in in_names
    ]


def _split_outputs(out_arrs, out_names, out_avals):
    results = []
    for c in range(N_CORES):
        results.append(
            {
                name: np.asarray(out_arrs[i]).reshape(
                    N_CORES, *out_avals[i].shape
                )[c]
                for i, name in enumerate(out_names)
            }
        )
    return results


def _const_inputs():
    xio = np.broadcast_to(
        np.arange(1, W + 1, dtype=np.float32)[None, :], (IMGS, W)
    ).copy()
    p = np.arange(128, dtype=np.float32)[:, None]
    c = np.arange(IMGS * NCHUNK, dtype=np.float32)[None, :] % NCHUNK
    yio = (c * 128.0 + p + 1.0).astype(np.float32)
    ident = np.eye(128, dtype=np.float32)
    return xio, yio, ident


def _in_maps_for(mask):
    xio, yio, ident = _const_inputs()
    in_maps = []
    for k in range(N_CORES):
        shard = np.ascontiguousarray(mask[k * BPC : (k + 1) * BPC])
        in_maps.append({"mask": shard, "xio": xio, "yio": yio, "ident": ident})
    return in_maps


def _extract(results):
    stats = np.stack(
        [results[k]["stats"].reshape(2, IMGS) for k in range(N_CORES)]
    )  # [cores, 2, IMGS]
    notall = stats[:, 0].reshape(B, C)
    ymax = stats[:, 1].reshape(B, C)
    xmax = np.stack(
        [results[k]["xmax"].reshape(IMGS) for k in range(N_CORES)]
    ).reshape(B, C)
    return notall, xmax, ymax


def run_device(mask, trace=False, **kw):
    """Runs the SPMD kernel; returns (not_all_on, x_max, y_max) as [B, C] f32
    arrays plus the per-core results."""
    if trace:
        nc = _get_nc()
        res = run_bass_kernel_spmd(
            nc, _in_maps_for(mask), core_ids=list(range(N_CORES)), trace=True, **kw
        )
        return (*_extract(res.results), res)
    sharded, mesh, in_names, out_names, out_avals, zero_outs = _make_runner()
    concat_in = _concat_inputs(_in_maps_for(mask), in_names)
    concat_zeros = [
        np.zeros((N_CORES * z.shape[0], *z.shape[1:]), z.dtype) for z in zero_outs
    ]
    out_arrs = sharded(*concat_in, *concat_zeros)
    results = _split_outputs(out_arrs, out_names, out_avals)
    return (*_extract(results), None)


def kernel(mask):
    mask = np.ascontiguousarray(np.asarray(mask, dtype=np.float32))
    assert mask.shape == (B, C, H, W), mask.shape
    notall, x_max, y_max, _ = run_device(mask)

    all_on = (notall == 0.0).astype(np.float32)
    x_min = all_on
    y_min = all_on
    object_found = ((y_max > y_min) & (x_max > x_min)).astype(np.int32)[..., None]
    boxes_scaled = np.stack([x_min, y_min, x_max, y_max], axis=-1).astype(np.float32)
    boxes = (boxes_scaled * SCALE).astype(np.float32)
    return object_found, boxes_scaled, boxes


# revision 38
# speedup vs baseline: 1528.0723x; 1.1486x over previous
"""Trainium2 Bass kernel for nn_BoundingBoxExtractor.

Math (from the reference): per (batch, channel) image, with
binary = (mask >= 0.5):
  x_min = y_min = 1 if ALL pixels on else 0
  x_max = (last column with any on pixel) + 1, or 0 if none
  y_max = (last row with any on pixel) + 1, or 0 if none

Device mapping (per core, 4 batches x 8 channels = 32 images of 512x512):
  - DMA each image as a [128, 4, 512] f32 tile (partition = row % 128).
  - DVE tensor_scalar(is_ge 0.5) per 128-row chunk -> binary bf16 tile AND,
    via accum_out(add), the per-row on-count rc. rc>=0.5 gives row any-on
    (y side), rc>=511.5 gives row all-on (the x_min/y_min test).
  - PE matmul ones[128,1]^T @ binary[128,512], accumulating the 4 chunks into
    a PSUM [1, 512] tile = per-column on-counts.
  - ScalarE copies each image's column counts to partition 0 of a wide
    staging row (engines are lane-locked); one SBUF->SBUF DMA redistributes
    to [32, 512] for batched post-processing.
  - Tail: batched DVE ops build a [128, 96] stats tile (notall / xmax-part /
    ymax-part per image); PE transpose + DVE max-reduce does the
    cross-partition max; a [96, 1] result goes out.

Scheduling notes (hardware sync-wait limits): DVE instructions can carry only
ONE sync wait, DMA ~2. 1-element DVE "probes" absorb the DMA-completion and
PE ticks before the real ops run, and mask_pool bufs=8 keeps each load-DMA's
WAW predecessor on the SAME DMA lane (8 lanes, round-robin) so no cross-lane
wait is needed.
"""

import sys

import numpy as np

try:
    import concourse.bass as bass  # noqa: F401
except ImportError:  # container default: repo lives in /opt/trn_rl_repo
    sys.path.insert(0, "/opt/trn_rl_repo")
    import concourse.bass as bass

import concourse.bacc as bacc
import concourse.tile as tile
from concourse import mybir
from concourse.bass_utils import run_bass_kernel_spmd

N_CORES = 8
B, C, H, W = 32, 8, 512, 512
BPC = B // N_CORES  # batches per core
IMGS = BPC * C  # images per core
NCHUNK = H // 128  # 128-row chunks per image
SCALE = 2.0

F32 = mybir.dt.float32
BF16 = mybir.dt.bfloat16


def build_bass(repeat=1):
    # Bacc (not raw Bass): its generate_event_semaphores pass splits
    # multi-semaphore waits to satisfy the 1-wait-per-instruction HW limit.
    # repeat>1 re-runs the whole main loop (identical results) — used only to
    # measure per-iteration device time by slope, since the axon tunnel hides
    # single-kernel wall time behind transfer costs.
    nc = bacc.Bacc("TRN2", target_bir_lowering=False, debug=False)

    mask = nc.dram_tensor("mask", [BPC, C, H, W], F32, kind="ExternalInput").ap()
    # x_iota[i, w] = w + 1
    xio = nc.dram_tensor("xio", [IMGS, W], F32, kind="ExternalInput").ap()
    # y_iota[p, j] = (j % NCHUNK) * 128 + p + 1  (global row index + 1)
    yio = nc.dram_tensor("yio", [128, IMGS * NCHUNK], F32, kind="ExternalInput").ap()
    ident = nc.dram_tensor("ident", [128, 128], F32, kind="ExternalInput").ap()
    # stats col: [0:IMGS) = not_all_on, [IMGS:2*IMGS) = y_max
    stats_out = nc.dram_tensor("stats", [2 * IMGS, 1], F32, kind="ExternalOutput").ap()
    xmax_out = nc.dram_tensor("xmax", [IMGS, 1], F32, kind="ExternalOutput").ap()

    imgs = mask.rearrange("b c h w -> (b c) h w")

    with tile.TileContext(nc) as tc:
        with (
            tc.tile_pool(name="consts", bufs=1) as consts,
            tc.tile_pool(name="mask_pool", bufs=8) as mask_pool,
            tc.tile_pool(name="bin_pool", bufs=4) as bin_pool,
            tc.tile_pool(name="acc", bufs=1) as acc,
            tc.tile_pool(name="psum_mm", bufs=7, space="PSUM") as psum_mm,
            tc.tile_pool(name="psum_t", bufs=1, space="PSUM") as psum_t,
        ):
            ones = consts.tile([128, 1], BF16)
            nc.vector.memset(ones, 1.0)
            xio_t = consts.tile([IMGS, W], F32)
            nc.sync.dma_start(out=xio_t, in_=xio)
            yio_t = consts.tile([128, IMGS * NCHUNK], F32)
            nc.sync.dma_start(out=yio_t, in_=yio)
            ident_t = consts.tile([128, 128], F32)
            nc.sync.dma_start(out=ident_t, in_=ident)
            rc = acc.tile([128, IMGS * NCHUNK], F32)  # per-row on-counts
            xwide = acc.tile([1, IMGS * W], F32)  # col counts staging (part 0)
            # tail tiles (filled incrementally per 8-image group so only the
            # final transpose+reduce+output remain serial at the end)
            GRP = 8
            xstack = acc.tile([IMGS, W], F32)
            stat = acc.tile([128, 2 * IMGS], F32)
            nr = acc.tile([128, IMGS * NCHUNK], F32)
            xon = acc.tile([IMGS, W], F32)
            xcon = acc.tile([IMGS, W], F32)
            xmax_t = acc.tile([IMGS, 1], F32)
            yon = acc.tile([128, IMGS * NCHUNK], F32)
            ymul = acc.tile([128, IMGS * NCHUNK], F32)

            def tail_group(g):
                i0, i1 = g * GRP, (g + 1) * GRP
                j0, j1 = i0 * NCHUNK, i1 * NCHUNK
                # x side: redistribute this group's column counts (compute on
                # them happens once at the end - compute-engine partition
                # bases must be 32-aligned, so per-8 slices are not legal)
                nc.sync.dma_start(
                    out=xstack[i0:i1, :], in_=xwide[0:1, i0 * W : i1 * W]
                )
                # not-all-on flags
                nc.vector.tensor_scalar(
                    out=nr[:, j0:j1],
                    in0=rc[:, j0:j1],
                    scalar1=float(W) - 0.5,
                    scalar2=None,
                    op0=mybir.AluOpType.is_le,
                )
                nc.vector.tensor_reduce(
                    out=stat[:, i0:i1],
                    in_=nr[:, j0:j1].rearrange("p (i c) -> p i c", c=NCHUNK),
                    axis=mybir.AxisListType.X,
                    op=mybir.AluOpType.max,
                )
                # y contribution
                nc.vector.tensor_scalar(
                    out=yon[:, j0:j1],
                    in0=rc[:, j0:j1],
                    scalar1=0.5,
                    scalar2=None,
                    op0=mybir.AluOpType.is_ge,
                )
                nc.vector.tensor_mul(
                    ymul[:, j0:j1], yon[:, j0:j1], yio_t[:, j0:j1]
                )
                nc.vector.tensor_reduce(
                    out=stat[:, IMGS + i0 : IMGS + i1],
                    in_=ymul[:, j0:j1].rearrange("p (i c) -> p i c", c=NCHUNK),
                    axis=mybir.AxisListType.X,
                    op=mybir.AluOpType.max,
                )

            ps_hist = []
            for i_rep in range(repeat * IMGS):
                i = i_rep % IMGS
                img = imgs[i].rearrange("(c p) w -> p c w", c=NCHUNK)
                xt = mask_pool.tile([128, NCHUNK, W], F32)
                nc.sync.dma_start(out=xt, in_=img)
                bt = bin_pool.tile([128, NCHUNK, W], BF16)
                for c in range(NCHUNK):
                    j = i * NCHUNK + c
                    nc.vector.tensor_scalar(
                        out=bt[:, c, :],
                        in0=xt[:, c, :],
                        scalar1=0.5,
                        scalar2=None,
                        op0=mybir.AluOpType.is_ge,
                        op1=mybir.AluOpType.add,
                        accum_out=rc[:, j : j + 1],
                    )
                ps = psum_mm.tile([1, W], F32)
                ps_hist.append(ps)  # noqa: list grows with repeats; unused
                for c in range(NCHUNK):
                    nc.tensor.matmul(
                        ps,
                        ones,
                        bt[:, c, :],
                        start=(c == 0),
                        stop=(c == NCHUNK - 1),
                    )
                nc.scalar.activation(
                    out=xwide[0:1, i * W : (i + 1) * W],
                    in_=ps,
                    func=mybir.ActivationFunctionType.Copy,
                )
                if (i_rep + 1) % GRP == 0 and i_rep < IMGS:
                    tail_group(i_rep // GRP)

            # ---- final (tiny) tail ----
            nc.vector.tensor_scalar(
                out=xon,
                in0=xstack,
                scalar1=0.5,
                scalar2=None,
                op0=mybir.AluOpType.is_ge,
            )
            nc.vector.tensor_mul(xcon, xon, xio_t)
            nc.vector.tensor_reduce(
                out=xmax_t,
                in_=xcon,
                axis=mybir.AxisListType.X,
                op=mybir.AluOpType.max,
            )
            # x_max is per-image ([IMGS, 1] on partitions 0..31) - it goes
            # straight out. notall/y_max need the cross-partition max:
            # transpose [128, 64] -> PSUM [64, 128], then free-dim max-reduce.
            nc.sync.dma_start(out=xmax_out, in_=xmax_t)
            pt = psum_t.tile([2 * IMGS, 128], F32)
            nc.tensor.transpose(pt, stat, ident_t)
            statr = acc.tile([2 * IMGS, 1], F32)
            nc.vector.tensor_reduce(
                out=statr,
                in_=pt,
                axis=mybir.AxisListType.X,
                op=mybir.AluOpType.max,
            )
            nc.sync.dma_start(out=stats_out, in_=statr)

    nc.finalize()
    return nc


_NC = {}


def _get_nc(repeat=1):
    if repeat not in _NC:
        _NC[repeat] = build_bass(repeat)
    return _NC[repeat]


_RUNNER = {}


def _make_runner(repeat=1):
    """Builds a reusable sharded-jit runner for the SPMD kernel (the same
    lowering run_bass_via_pjrt does, but cached so repeat calls reuse the
    compiled executable and device-resident inputs are possible)."""
    if repeat in _RUNNER:
        return _RUNNER[repeat]
    import jax
    from concourse import bass2jax, mybir as mb

    nc = _get_nc(repeat)
    bass2jax.install_neuronx_cc_hook()
    partition_name = nc.partition_id_tensor.name if nc.partition_id_tensor else None
    in_names, out_names, out_avals, zero_outs = [], [], [], []
    for alloc in nc.m.functions[0].allocations:
        if not isinstance(alloc, mb.MemoryLocationSet):
            continue
        name = alloc.memorylocations[0].name
        if alloc.kind == "ExternalInput":
            if name != partition_name:
                in_names.append(name)
        elif alloc.kind == "ExternalOutput":
            out_names.append(name)
            shape = tuple(alloc.tensor_shape)
            dtype = mb.dt.np(alloc.dtype)
            out_avals.append(jax.core.ShapedArray(shape, dtype))
            zero_outs.append(np.zeros(shape, dtype))
    n_params = len(in_names)
    all_in_names = list(in_names) + list(out_names)
    if partition_name is not None:
        all_in_names.append(partition_name)
    donate = tuple(range(n_params, n_params + len(out_names)))

    def _body(*args):
        operands = list(args)
        if partition_name is not None:
            operands.append(bass2jax.partition_id_tensor())
        outs = bass2jax._bass_exec_p.bind(
            *operands,
            out_avals=tuple(out_avals),
            in_names=tuple(all_in_names),
            out_names=tuple(out_names),
            lowering_input_output_aliases=(),
            sim_require_finite=True,
            sim_require_nnan=True,
            nc=nc,
        )
        return tuple(outs)

    devices = jax.devices()[:N_CORES]
    mesh = bass2jax.Mesh(np.asarray(devices), ("core",))
    P = bass2jax.PartitionSpec
    sharded = jax.jit(
        bass2jax.shard_map(
            _body,
            mesh=mesh,
            in_specs=(P("core"),) * (n_params + len(out_names)),
            out_specs=(P("core"),) * len(out_names),
            check_rep=False,
        ),
        donate_argnums=donate,
        keep_unused=True,
    )
    _RUNNER[repeat] = (sharded, mesh, in_names, out_names, out_avals, zero_outs)
    return _RUNNER[repeat]


def _concat_inputs(in_maps, in_names):
    return [
        np.concatenate([np.asarray(m[name]) for m in in_maps], axis=0)
        for name in in_names
    ]


def _split_outputs(out_arrs, out_names, out_avals):
    results = []
    for c in range(N_CORES):
        results.append(
            {
                name: np.asarray(out_arrs[i]).reshape(
                    N_CORES, *out_avals[i].shape
                )[c]
                for i, name in enumerate(out_names)
            }
        )
    return results


def _const_inputs():
    xio = np.broadcast_to(
        np.arange(1, W + 1, dtype=np.float32)[None, :], (IMGS, W)
    ).copy()
    p = np.arange(128, dtype=np.float32)[:, None]
    c = np.arange(IMGS * NCHUNK, dtype=np.float32)[None, :] % NCHUNK
    yio = (c * 128.0 + p + 1.0).astype(np.float32)
    ident = np.eye(128, dtype=np.float32)
    return xio, yio, ident


def _in_maps_for(mask):
    xio, yio, ident = _const_inputs()
    in_maps = []
    for k in range(N_CORES):
        shard = np.ascontiguousarray(mask[k * BPC : (k + 1) * BPC])
        in_maps.append({"mask": shard, "xio": xio, "yio": yio, "ident": ident})
    return in_maps


def _extract(results):
    stats = np.stack(
        [results[k]["stats"].reshape(2, IMGS) for k in range(N_CORES)]
    )  # [cores, 2, IMGS]
    notall = stats[:, 0].reshape(B, C)
    ymax = stats[:, 1].reshape(B, C)
    xmax = np.stack(
        [results[k]["xmax"].reshape(IMGS) for k in range(N_CORES)]
    ).reshape(B, C)
    return notall, xmax, ymax


def run_device(mask, trace=False, **kw):
    """Runs the SPMD kernel; returns (not_all_on, x_max, y_max) as [B, C] f32
    arrays plus the per-core results."""
    if trace:
        nc = _get_nc()
        res = run_bass_kernel_spmd(
            nc, _in_maps_for(mask), core_ids=list(range(N_CORES)), trace=True, **kw
        )
        return (*_extract(res.results), res)
    sharded, mesh, in_names, out_names, out_avals, zero_outs = _make_runner()
    concat_in = _concat_inputs(_in_maps_for(mask), in_names)
    concat_zeros = [
        np.zeros((N_CORES * z.shape[0], *z.shape[1:]), z.dtype) for z in zero_outs
    ]
    out_arrs = sharded(*concat_in, *concat_zeros)
    results = _split_outputs(out_arrs, out_names, out_avals)
    return (*_extract(results), None)


def kernel(mask):
    mask = np.ascontiguousarray(np.asarray(mask, dtype=np.float32))
    assert mask.shape == (B, C, H, W), mask.shape
    notall, x_max, y_max, _ = run_device(mask)

    all_on = (notall == 0.0).astype(np.float32)
    x_min = all_on
    y_min = all_on
    object_found = ((y_max > y_min) & (x_max > x_min)).astype(np.int32)[..., None]
    boxes_scaled = np.stack([x_min, y_min, x_max, y_max], axis=-1).astype(np.float32)
    boxes = (boxes_scaled * SCALE).astype(np.float32)
    return object_found, boxes_scaled, boxes


# revision 42
# speedup vs baseline: 6397.4873x; 4.1866x over previous
"""Trainium2 Bass kernel for nn_BoundingBoxExtractor.

Math (from the reference): per (batch, channel) image, with
binary = (mask >= 0.5):
  x_min = y_min = 1 if ALL pixels on else 0
  x_max = (last column with any on pixel) + 1, or 0 if none
  y_max = (last row with any on pixel) + 1, or 0 if none

Device mapping (per core, 4 batches x 8 channels = 32 images of 512x512):
  - DMA each image as a [128, 4, 512] f32 tile (partition = row % 128).
  - DVE tensor_scalar(is_ge 0.5) per 128-row chunk -> binary bf16 tile AND,
    via accum_out(add), the per-row on-count rc. rc>=0.5 gives row any-on
    (y side), rc>=511.5 gives row all-on (the x_min/y_min test).
  - PE matmul ones[128,1]^T @ binary[128,512], accumulating the 4 chunks into
    a PSUM [1, 512] tile = per-column on-counts.
  - ScalarE copies each image's column counts to partition 0 of a wide
    staging row (engines are lane-locked); one SBUF->SBUF DMA redistributes
    to [32, 512] for batched post-processing.
  - Tail: batched DVE ops build a [128, 96] stats tile (notall / xmax-part /
    ymax-part per image); PE transpose + DVE max-reduce does the
    cross-partition max; a [96, 1] result goes out.

Scheduling notes (hardware sync-wait limits): DVE instructions can carry only
ONE sync wait, DMA ~2. 1-element DVE "probes" absorb the DMA-completion and
PE ticks before the real ops run, and mask_pool bufs=8 keeps each load-DMA's
WAW predecessor on the SAME DMA lane (8 lanes, round-robin) so no cross-lane
wait is needed.
"""

import sys

import numpy as np

try:
    import concourse.bass as bass  # noqa: F401
except ImportError:  # container default: repo lives in /opt/trn_rl_repo
    sys.path.insert(0, "/opt/trn_rl_repo")
    import concourse.bass as bass

import concourse.bacc as bacc
import concourse.tile as tile
from concourse import mybir
from concourse.bass_utils import run_bass_kernel_spmd

N_CORES = 8
B, C, H, W = 32, 8, 512, 512
BPC = B // N_CORES  # batches per core
IMGS = BPC * C  # images per core
NCHUNK = H // 128  # 128-row chunks per image
SCALE = 2.0

F32 = mybir.dt.float32
BF16 = mybir.dt.bfloat16


def build_bass(repeat=1):
    # Bacc (not raw Bass): its generate_event_semaphores pass splits
    # multi-semaphore waits to satisfy the 1-wait-per-instruction HW limit.
    # repeat>1 re-runs the whole main loop (identical results) — used only to
    # measure per-iteration device time by slope, since the axon tunnel hides
    # single-kernel wall time behind transfer costs.
    nc = bacc.Bacc("TRN2", target_bir_lowering=False, debug=False)

    # The device only needs the top 16 bits of each f32: for x in [0, 1),
    # (x >= 0.5) == (truncate_to_bf16(x) >= 0.5) exactly (truncation is
    # monotone and 0.5 is representable), so the host ships bf16 halves and
    # device DMA traffic halves.
    mask = nc.dram_tensor("mask", [BPC, C, H, W], BF16, kind="ExternalInput").ap()
    # x_iota[i, w] = w + 1
    xio = nc.dram_tensor("xio", [IMGS, W], F32, kind="ExternalInput").ap()
    # y_iota[p, j] = (j % NCHUNK) * 128 + p + 1  (global row index + 1)
    yio = nc.dram_tensor("yio", [128, IMGS * NCHUNK], F32, kind="ExternalInput").ap()
    ident = nc.dram_tensor("ident", [128, 128], F32, kind="ExternalInput").ap()
    # stats col: [0:IMGS) = not_all_on, [IMGS:2*IMGS) = y_max
    stats_out = nc.dram_tensor("stats", [2 * IMGS, 1], F32, kind="ExternalOutput").ap()
    xmax_out = nc.dram_tensor("xmax", [IMGS, 1], F32, kind="ExternalOutput").ap()

    imgs = mask.rearrange("b c h w -> (b c) h w")

    with tile.TileContext(nc) as tc:
        with (
            tc.tile_pool(name="consts", bufs=1) as consts,
            tc.tile_pool(name="mask_pool", bufs=8) as mask_pool,
            tc.tile_pool(name="bin_pool", bufs=4) as bin_pool,
            tc.tile_pool(name="acc", bufs=1) as acc,
            tc.tile_pool(name="psum_mm", bufs=7, space="PSUM") as psum_mm,
            tc.tile_pool(name="psum_t", bufs=1, space="PSUM") as psum_t,
        ):
            ones = consts.tile([128, 1], BF16)
            nc.vector.memset(ones, 1.0)
            xio_t = consts.tile([IMGS, W], F32)
            nc.sync.dma_start(out=xio_t, in_=xio)
            yio_t = consts.tile([128, IMGS * NCHUNK], F32)
            nc.sync.dma_start(out=yio_t, in_=yio)
            ident_t = consts.tile([128, 128], F32)
            nc.sync.dma_start(out=ident_t, in_=ident)
            rc = acc.tile([128, IMGS * NCHUNK], F32)  # per-row on-counts
            xwide = acc.tile([1, IMGS * W], F32)  # col counts staging (part 0)
            # tail tiles (filled incrementally per 8-image group so only the
            # final transpose+reduce+output remain serial at the end)
            GRP = 8
            xstack = acc.tile([IMGS, W], F32)
            stat = acc.tile([128, 2 * IMGS], F32)
            nr = acc.tile([128, IMGS * NCHUNK], F32)
            xon = acc.tile([IMGS, W], F32)
            xcon = acc.tile([IMGS, W], F32)
            xmax_t = acc.tile([IMGS, 1], F32)
            yon = acc.tile([128, IMGS * NCHUNK], F32)
            ymul = acc.tile([128, IMGS * NCHUNK], F32)

            def tail_group(g):
                i0, i1 = g * GRP, (g + 1) * GRP
                j0, j1 = i0 * NCHUNK, i1 * NCHUNK
                # x side: redistribute this group's column counts (compute on
                # them happens once at the end - compute-engine partition
                # bases must be 32-aligned, so per-8 slices are not legal)
                nc.sync.dma_start(
                    out=xstack[i0:i1, :], in_=xwide[0:1, i0 * W : i1 * W]
                )
                # not-all-on flags
                nc.vector.tensor_scalar(
                    out=nr[:, j0:j1],
                    in0=rc[:, j0:j1],
                    scalar1=float(W) - 0.5,
                    scalar2=None,
                    op0=mybir.AluOpType.is_le,
                )
                nc.vector.tensor_reduce(
                    out=stat[:, i0:i1],
                    in_=nr[:, j0:j1].rearrange("p (i c) -> p i c", c=NCHUNK),
                    axis=mybir.AxisListType.X,
                    op=mybir.AluOpType.max,
                )
                # y contribution
                nc.vector.tensor_scalar(
                    out=yon[:, j0:j1],
                    in0=rc[:, j0:j1],
                    scalar1=0.5,
                    scalar2=None,
                    op0=mybir.AluOpType.is_ge,
                )
                nc.vector.tensor_mul(
                    ymul[:, j0:j1], yon[:, j0:j1], yio_t[:, j0:j1]
                )
                nc.vector.tensor_reduce(
                    out=stat[:, IMGS + i0 : IMGS + i1],
                    in_=ymul[:, j0:j1].rearrange("p (i c) -> p i c", c=NCHUNK),
                    axis=mybir.AxisListType.X,
                    op=mybir.AluOpType.max,
                )

            ps_hist = []
            for i_rep in range(repeat * IMGS):
                i = i_rep % IMGS
                img = imgs[i].rearrange("(c p) w -> p c w", c=NCHUNK)
                xt = mask_pool.tile([128, NCHUNK, W], BF16)
                nc.sync.dma_start(out=xt, in_=img)
                bt = bin_pool.tile([128, NCHUNK, W], BF16)
                for c in range(NCHUNK):
                    j = i * NCHUNK + c
                    nc.vector.tensor_scalar(
                        out=bt[:, c, :],
                        in0=xt[:, c, :],
                        scalar1=0.5,
                        scalar2=None,
                        op0=mybir.AluOpType.is_ge,
                        op1=mybir.AluOpType.add,
                        accum_out=rc[:, j : j + 1],
                    )
                ps = psum_mm.tile([1, W], F32)
                ps_hist.append(ps)  # noqa: list grows with repeats; unused
                for c in range(NCHUNK):
                    nc.tensor.matmul(
                        ps,
                        ones,
                        bt[:, c, :],
                        start=(c == 0),
                        stop=(c == NCHUNK - 1),
                    )
                nc.scalar.activation(
                    out=xwide[0:1, i * W : (i + 1) * W],
                    in_=ps,
                    func=mybir.ActivationFunctionType.Copy,
                )
                if (i_rep + 1) % GRP == 0 and i_rep < IMGS:
                    tail_group(i_rep // GRP)

            # ---- final (tiny) tail ----
            nc.vector.tensor_scalar(
                out=xon,
                in0=xstack,
                scalar1=0.5,
                scalar2=None,
                op0=mybir.AluOpType.is_ge,
            )
            nc.vector.tensor_mul(xcon, xon, xio_t)
            nc.vector.tensor_reduce(
                out=xmax_t,
                in_=xcon,
                axis=mybir.AxisListType.X,
                op=mybir.AluOpType.max,
            )
            # x_max is per-image ([IMGS, 1] on partitions 0..31) - it goes
            # straight out. notall/y_max need the cross-partition max:
            # transpose [128, 64] -> PSUM [64, 128], then free-dim max-reduce.
            nc.sync.dma_start(out=xmax_out, in_=xmax_t)
            pt = psum_t.tile([2 * IMGS, 128], F32)
            nc.tensor.transpose(pt, stat, ident_t)
            statr = acc.tile([2 * IMGS, 1], F32)
            nc.vector.tensor_reduce(
                out=statr,
                in_=pt,
                axis=mybir.AxisListType.X,
                op=mybir.AluOpType.max,
            )
            nc.sync.dma_start(out=stats_out, in_=statr)

    nc.finalize()
    return nc


_NC = {}


def _get_nc(repeat=1):
    if repeat not in _NC:
        _NC[repeat] = build_bass(repeat)
    return _NC[repeat]


_RUNNER = {}


def _make_runner(repeat=1):
    """Builds a reusable sharded-jit runner for the SPMD kernel (the same
    lowering run_bass_via_pjrt does, but cached so repeat calls reuse the
    compiled executable and device-resident inputs are possible)."""
    if repeat in _RUNNER:
        return _RUNNER[repeat]
    import jax
    from concourse import bass2jax, mybir as mb

    nc = _get_nc(repeat)
    bass2jax.install_neuronx_cc_hook()
    partition_name = nc.partition_id_tensor.name if nc.partition_id_tensor else None
    in_names, out_names, out_avals, zero_outs = [], [], [], []
    for alloc in nc.m.functions[0].allocations:
        if not isinstance(alloc, mb.MemoryLocationSet):
            continue
        name = alloc.memorylocations[0].name
        if alloc.kind == "ExternalInput":
            if name != partition_name:
                in_names.append(name)
        elif alloc.kind == "ExternalOutput":
            out_names.append(name)
            shape = tuple(alloc.tensor_shape)
            dtype = mb.dt.np(alloc.dtype)
            out_avals.append(jax.core.ShapedArray(shape, dtype))
            zero_outs.append(np.zeros(shape, dtype))
    n_params = len(in_names)
    all_in_names = list(in_names) + list(out_names)
    if partition_name is not None:
        all_in_names.append(partition_name)
    donate = tuple(range(n_params, n_params + len(out_names)))

    def _body(*args):
        operands = list(args)
        if partition_name is not None:
            operands.append(bass2jax.partition_id_tensor())
        outs = bass2jax._bass_exec_p.bind(
            *operands,
            out_avals=tuple(out_avals),
            in_names=tuple(all_in_names),
            out_names=tuple(out_names),
            lowering_input_output_aliases=(),
            sim_require_finite=True,
            sim_require_nnan=True,
            nc=nc,
        )
        return tuple(outs)

    devices = jax.devices()[:N_CORES]
    mesh = bass2jax.Mesh(np.asarray(devices), ("core",))
    P = bass2jax.PartitionSpec
    sharded = jax.jit(
        bass2jax.shard_map(
            _body,
            mesh=mesh,
            in_specs=(P("core"),) * (n_params + len(out_names)),
            out_specs=(P("core"),) * len(out_names),
            check_rep=False,
        ),
        donate_argnums=donate,
        keep_unused=True,
    )
    _RUNNER[repeat] = (sharded, mesh, in_names, out_names, out_avals, zero_outs)
    return _RUNNER[repeat]


def _concat_inputs(in_maps, in_names):
    return [
        np.concatenate([np.asarray(m[name]) for m in in_maps], axis=0)
        for name in in_names
    ]


def _split_outputs(out_arrs, out_names, out_avals):
    results = []
    for c in range(N_CORES):
        results.append(
            {
                name: np.asarray(out_arrs[i]).reshape(
                    N_CORES, *out_avals[i].shape
                )[c]
                for i, name in enumerate(out_names)
            }
        )
    return results


def _const_inputs():
    xio = np.broadcast_to(
        np.arange(1, W + 1, dtype=np.float32)[None, :], (IMGS, W)
    ).copy()
    p = np.arange(128, dtype=np.float32)[:, None]
    c = np.arange(IMGS * NCHUNK, dtype=np.float32)[None, :] % NCHUNK
    yio = (c * 128.0 + p + 1.0).astype(np.float32)
    ident = np.eye(128, dtype=np.float32)
    return xio, yio, ident


def _to_bf16_trunc(mask_f32):
    """Top 16 bits of each f32, viewed as bf16 (little-endian high half)."""
    import ml_dtypes

    hi = mask_f32.view(np.uint16).reshape(*mask_f32.shape, 2)[..., 1]
    return np.ascontiguousarray(hi).view(ml_dtypes.bfloat16)


def _in_maps_for(mask):
    xio, yio, ident = _const_inputs()
    mask16 = _to_bf16_trunc(np.ascontiguousarray(mask))
    in_maps = []
    for k in range(N_CORES):
        shard = np.ascontiguousarray(mask16[k * BPC : (k + 1) * BPC])
        in_maps.append({"mask": shard, "xio": xio, "yio": yio, "ident": ident})
    return in_maps


def _extract(results):
    stats = np.stack(
        [results[k]["stats"].reshape(2, IMGS) for k in range(N_CORES)]
    )  # [cores, 2, IMGS]
    notall = stats[:, 0].reshape(B, C)
    ymax = stats[:, 1].reshape(B, C)
    xmax = np.stack(
        [results[k]["xmax"].reshape(IMGS) for k in range(N_CORES)]
    ).reshape(B, C)
    return notall, xmax, ymax


def run_device(mask, trace=False, **kw):
    """Runs the SPMD kernel; returns (not_all_on, x_max, y_max) as [B, C] f32
    arrays plus the per-core results."""
    if trace:
        nc = _get_nc()
        res = run_bass_kernel_spmd(
            nc, _in_maps_for(mask), core_ids=list(range(N_CORES)), trace=True, **kw
        )
        return (*_extract(res.results), res)
    sharded, mesh, in_names, out_names, out_avals, zero_outs = _make_runner()
    # concat-over-cores of the batch-sharded mask is just the full array;
    # build it directly instead of split+reconcat (saves ~256 MB of memcpy)
    xio, yio, ident = _const_inputs()
    mask16 = _to_bf16_trunc(np.ascontiguousarray(mask))
    per_name = {
        "mask": mask16,
        "xio": np.tile(xio, (N_CORES, 1)),
        "yio": np.tile(yio, (N_CORES, 1)),
        "ident": np.tile(ident, (N_CORES, 1)),
    }
    concat_in = [per_name[name] for name in in_names]
    concat_zeros = [
        np.zeros((N_CORES * z.shape[0], *z.shape[1:]), z.dtype) for z in zero_outs
    ]
    out_arrs = sharded(*concat_in, *concat_zeros)
    results = _split_outputs(out_arrs, out_names, out_avals)
    return (*_extract(results), None)


def kernel(mask):
    mask = np.ascontiguousarray(np.asarray(mask, dtype=np.float32))
    assert mask.shape == (B, C, H, W), mask.shape
    notall, x_max, y_max, _ = run_device(mask)

    all_on = (notall == 0.0).astype(np.float32)
    x_min = all_on
    y_min = all_on
    object_found = ((y_max > y_min) & (x_max > x_min)).astype(np.int32)[..., None]
    boxes_scaled = np.stack([x_min, y_min, x_max, y_max], axis=-1).astype(np.float32)
    boxes = (boxes_scaled * SCALE).astype(np.float32)
    return object_found, boxes_scaled, boxes
